# revision 1
# baseline (speedup 1.0000x reference)
"""Causal self-attention (B=2, S=2048, D=1024, H=16, Hd=64) on 8 TRN2 NeuronCores.

Sharding: tensor-parallel over heads (4 heads/core) x data-parallel over batch
(cores 0-3 -> batch 0, cores 4-7 -> batch 1). Each core:
  - computes q^T,k^T (transposed layout, heads stacked in pairs on partitions)
    and v (natural layout) for its 4 heads
  - runs causal attention in transposed-score layout (scores_T[k, q]) so no
    transposes are ever needed; softmax denominators come for free from a
    ones-column appended to V; normalization via fast-approx DVE reciprocal +
    gpsimd partition_broadcast
  - computes its partial output projection y_part = out_heads @ W_proj[rows]
Host sums the 4 bf16 partials per batch and adds b_proj (the unshard step for
a row-parallel matmul). Matmul datapath is bf16 (fp32 PSUM accumulation).

Schedule: software-pipelined. The qkv-projection / v / output-projection
matmuls are split into quarter-granular "filler" units that are interleaved
between attention steps, so the PE stays busy while ScalarE chews the exp
stream (the attention inner loop is activation-bound). Non-exp evictions run
on DVE, keeping ScalarE for exp only.
"""

import sys

if "/opt/trn_rl_repo" not in sys.path:
    sys.path.insert(0, "/opt/trn_rl_repo")

import ml_dtypes
import numpy as np


def _ensure_axon_hooks():
    """bass_utils imports antenv.axon_hooks when tracing is requested; the
    slim agent image lacks it. Provide the real ctypes hook if possible,
    else a None-returning stub (bass_utils then skips tracing gracefully)."""
    try:
        import antenv.axon_hooks  # noqa: F401

        return
    except ImportError:
        pass
    import types

    hook = None
    try:
        from trn_agent_boot.trn_boot import _ntff_profile_via_ctypes

        hook = _ntff_profile_via_ctypes("/opt/axon/libaxon_pjrt.so")
    except Exception:
        pass
    mod = types.ModuleType("antenv.axon_hooks")
    mod.get_axon_ntff_profile_hook = lambda: hook
    mod.set_axon_ntff_profile_hook = lambda h: None
    sys.modules["antenv.axon_hooks"] = mod


_ensure_axon_hooks()

D = 1024
S = 2048
B = 2
H = 16
HD = 64
N_CORES = 8
GROUPS = 4  # cores per batch
HPC = 4  # heads per core
SCALE = 1.0 / np.sqrt(HD)
KT = D // 128  # 8 contraction tiles
ST = S // 128  # 16 seq tiles

_module_cache = {}


def _build_module():
    if "nc" in _module_cache:
        return _module_cache["nc"]

    import concourse.bacc as bacc
    import concourse.mybir as mybir
    import concourse.tile as tile
    from concourse.bass import ts

    f32 = mybir.dt.float32
    bf16 = mybir.dt.bfloat16
    AF = mybir.ActivationFunctionType

    nc = bacc.Bacc("TRN2", target_bir_lowering=False, debug=False)

    xT = nc.dram_tensor("xT", [D, S], bf16, kind="ExternalInput")
    w_qk = nc.dram_tensor("w_qk", [D, 512], bf16, kind="ExternalInput")
    b_qk = nc.dram_tensor("b_qk", [128, 4], f32, kind="ExternalInput")
    w_v = nc.dram_tensor("w_v", [D, 256], bf16, kind="ExternalInput")
    b_v = nc.dram_tensor("b_v", [128, 256], f32, kind="ExternalInput")
    w_pr = nc.dram_tensor("w_pr", [256, D], bf16, kind="ExternalInput")
    y = nc.dram_tensor("y", [S, D], bf16, kind="ExternalOutput")

    import contextlib

    with tile.TileContext(nc) as tc:
        with contextlib.ExitStack() as ctx2:
            const = ctx2.enter_context(tc.tile_pool(name="const", bufs=1))
            # ---- resident SBUF tensors ----
            xT_sb = const.tile([128, KT, S], bf16)
            wqk_sb = const.tile([128, KT, 512], bf16)
            wv_sb = const.tile([128, KT, 256], bf16)
            bqk_sb = const.tile([128, 4], f32)
            bv_sb = const.tile([128, 4, 64], f32)
            wpr_sb = const.tile([128, 2, D], bf16)
            ones_sb = const.tile([1, 64], f32)
            warm_sb = const.tile([1, 64], f32)
            qkT_sb = const.tile([128, 4, S], bf16)  # m: q01,q23,k01,k23
            wu_sb = const.tile([128, 512], bf16)  # HAM warmup operand
            v_sb = const.tile([128, ST, 4, 128], bf16)  # per head: [ones|63 pad|V]
            oT_sb = const.tile([128, 2, S], bf16)  # normalized attn out

            # The input load is HBM-bound (~250-300GB/s aggregate); two
            # queues (sync + scalar) carry the wqk/xT stream that gates the
            # first scores, with wv split right behind it on both queues and
            # wpr last — strict priority by queue order. Issue rate is
            # ~0.6us/DMA per queue, so splitting also halves issue latency.
            nc.vector.memset(ones_sb[:], 1.0)
            nc.vector.memset(wu_sb[:], 0.03)
            # preload the ACT exp table set early, off the critical path
            nc.scalar.activation(warm_sb[:], ones_sb[:], AF.Exp)
            nc.sync.dma_start(out=bqk_sb[:], in_=b_qk[:])
            nc.sync.dma_start(out=bv_sb[:], in_=b_v[:])
            for k in range(KT):
                nc.sync.dma_start(out=wqk_sb[:, k, :], in_=w_qk[ts(k, 128), :])
                nc.sync.dma_start(out=xT_sb[:, k, :], in_=xT[ts(k, 128), :])
            for k in range(KT):
                nc.sync.dma_start(out=wv_sb[:, k, :], in_=w_v[ts(k, 128), :])
            nc.sync.dma_start(out=wpr_sb[:, 0, :], in_=w_pr[0:128, :])
            nc.sync.dma_start(out=wpr_sb[:, 1, :], in_=w_pr[128:256, :])
            for h in range(HPC):
                # ones in column 0: the denominator lands on PARTITION 0 of
                # the attnV psum (reciprocal_approx_fast and
                # partition_broadcast require base partition 0); cols 1-63
                # pad V outputs to the upper 64 PSUM partitions (PSUM
                # reads must be 0- or 64-based). On gpsimd: the DVE queue
                # must stay clear for the ramp evictions.
                nc.gpsimd.memset(v_sb[:, :, h, 0:1], 1.0)
                nc.gpsimd.memset(v_sb[:, :, h, 1:64], 0.0)

            # PSUM budget (8 banks): scores 2 bufs x [128,2,512] = 4 banks,
            # attnV accumulators 2 x [65,512] = 2 banks, filler [128,1024]
            # = 2 banks.
            psS = ctx2.enter_context(tc.tile_pool(name="psS", bufs=2, space="PSUM"))
            psO = ctx2.enter_context(tc.tile_pool(name="psO", bufs=1, space="PSUM"))
            psF = ctx2.enter_context(tc.tile_pool(name="psF", bufs=2, space="PSUM"))
            ptp = ctx2.enter_context(tc.tile_pool(name="pt", bufs=6))
            ysbp = ctx2.enter_context(tc.tile_pool(name="ysb", bufs=3))
            nrm = ctx2.enter_context(tc.tile_pool(name="nrm", bufs=2))

            # ---- filler units: generators yielding approx PE-ns per matmul
            def gen_qk_quarter(m, q2):
                """qkT_sb[:, m, q2*512:...] = (x @ w_qk[:, m-tile]) + bias."""
                ps = psF.tile([128, 512], f32, tag="f", name="ps_qk")
                for k in range(KT):
                    nc.tensor.matmul(
                        ps[:, 0:512],
                        lhsT=wqk_sb[:, k, ts(m, 128)],
                        rhs=xT_sb[:, k, ts(q2, 512)],
                        start=(k == 0),
                        stop=(k == KT - 1),
                    )
                    yield 216.0
                nc.vector.tensor_scalar_add(
                    qkT_sb[:, m, ts(q2, 512)], ps[:, 0:512], bqk_sb[:, m : m + 1]
                )
                yield 0.0

            def gen_v_quarter(qt):
                """v_sb seq-tiles 2qt, 2qt+1 (natural layout, + bias)."""
                ps = psF.tile([128, 2, 4, 64], f32, tag="f", name="ps_v")
                for sti in range(2):
                    st = qt * 2 + sti
                    for k in range(KT):
                        nc.tensor.matmul(
                            ps[:, sti],
                            lhsT=xT_sb[:, k, ts(st, 128)],
                            rhs=wv_sb[:, k, :],
                            start=(k == 0),
                            stop=(k == KT - 1),
                        )
                        yield 110.0
                    nc.vector.tensor_add(
                        v_sb[:, st, :, 64:128], ps[:, sti], bv_sb[:]
                    )
                    yield 0.0

            def gen_proj(m, nch):
                """y[m-tile, nch half] = oT[m-tile]^T @ w_pr[:, nch half]."""
                ps = psF.tile([128, 512], f32, tag="f", name="ps_y")
                for kp in range(2):
                    nc.tensor.matmul(
                        ps[:, 0:512],
                        lhsT=oT_sb[:, kp, ts(m, 128)],
                        rhs=wpr_sb[:, kp, ts(nch, 512)],
                        start=(kp == 0),
                        stop=(kp == 1),
                    )
                    yield 216.0
                y_sb = ysbp.tile([128, 512], bf16, tag="ysb", name="y_sb")
                nc.vector.tensor_scalar_add(y_sb[:], ps[:], 0.0)
                nc.sync.dma_start(
                    out=y[ts(m, 128), ts(nch, 512)], in_=y_sb[:]
                )
                yield 0.0

            class Filler:
                """Queue of named filler units (generators of PE matmuls).
                pull(ns) paces emission by approximate PE time; ensure(key)
                force-emits a unit NOW (producers must be emitted before
                their consumers — units are mutually independent)."""

                def __init__(self):
                    self.queue = []  # list of (key, gen)
                    self.cur = None  # (key, gen) partially emitted

                def add(self, key, gen):
                    self.queue.append((key, gen))

                def ensure(self, key):
                    if self.cur is not None and self.cur[0] == key:
                        for _ in self.cur[1]:
                            pass
                        self.cur = None
                        return
                    for i, (k, g) in enumerate(self.queue):
                        if k == key:
                            del self.queue[i]
                            for _ in g:
                                pass
                            return

                def pull(self, ns):
                    while ns > 0:
                        if self.cur is None:
                            if not self.queue:
                                return
                            self.cur = self.queue.pop(0)
                        try:
                            ns -= next(self.cur[1])
                        except StopIteration:
                            self.cur = None

                def drain(self):
                    while self.cur is not None or self.queue:
                        self.pull(1e12)

            filler = Filler()

            # ---- attention chunk: 512 q-cols of one head pair ----
            def attn_chunk(hp, jj, pull_scale, prefetch=()):
                c0 = 512 * jj
                n_sk = 4 * jj + 4
                po = [
                    psO.tile([128, 512], f32, tag=f"o{h}", name=f"po{h}")
                    for h in range(2)
                ]

                def emit_attnv(sk, pt, col0, n):
                    for h in range(2):
                        nc.tensor.matmul(
                            po[h][:, col0 - c0 : col0 - c0 + n],
                            lhsT=v_sb[:, sk, 2 * hp + h, :],
                            rhs=pt[:, h, 0:n],
                            start=(sk == 0),
                            stop=(sk == n_sk - 1),
                        )

                filler.ensure(("qk", hp, jj))  # this chunk's q quarter
                prev = None
                for sk in range(n_sk):
                    col0 = max(c0, sk * 128)
                    n = c0 + 512 - col0
                    filler.ensure(("qk", 2 + hp, sk // 4))  # k quarter
                    if sk == 2:
                        for key in prefetch:
                            filler.ensure(key)
                    ps = psS.tile([128, 2, 512], f32, tag="s", name="ps")
                    for h in range(2):
                        hr = h * 64
                        nc.tensor.matmul(
                            ps[:, h, 0:n],
                            lhsT=qkT_sb[hr : hr + 64, 2 + hp, ts(sk, 128)],
                            rhs=qkT_sb[hr : hr + 64, hp, col0 : col0 + n],
                            start=True,
                            stop=True,
                            tile_position=(hr, 0),
                        )
                    pt = ptp.tile([128, 2, 512], bf16, tag="pt", name="pt")
                    if n == 512:
                        nc.scalar.activation(pt[:], ps[:], AF.Exp)
                    else:
                        nc.scalar.activation(pt[:, :, 0:n], ps[:, :, 0:n], AF.Exp)
                    if col0 == sk * 128:
                        # causal mask: zero pt below the diagonal on the
                        # (idle) gpsimd engine; the one-step delay before
                        # attnV consumes pt hides the latency, and it keeps
                        # the 64 mask matmuls off the bottleneck PE.
                        for h in range(2):
                            nc.gpsimd.affine_select(
                                out=pt[:, h, 0:128],
                                in_=pt[:, h, 0:128],
                                compare_op=mybir.AluOpType.is_ge,
                                fill=0.0,
                                base=0,
                                pattern=[[1, 128]],
                                channel_multiplier=-1,
                            )
                    # filler between this step's scores and the previous
                    # step's attnV: covers the exp latency on the PE queue.
                    act_ns = (2 * n + 352) / 1.2
                    attn_pe = (3 * n) / 2.4 + 60
                    filler.pull(max(150.0, (act_ns - attn_pe) * pull_scale))
                    if prev is not None:
                        filler.ensure(("v", prev[0] // 2))
                        emit_attnv(*prev)
                    prev = (sk, pt, col0, n)
                filler.ensure(("v", prev[0] // 2))
                emit_attnv(*prev)
                # normalization: rec = 1/denominator, broadcast, scale
                for h in range(2):
                    hr = h * 64
                    rec = nrm.tile([1, 512], f32, tag="rec", name="rec")
                    nc.vector.reciprocal_approx_fast(
                        out=rec[:], in_=po[h][0:1, :]
                    )
                    rbc = nrm.tile([64, 512], f32, tag="rbc", name="rbc")
                    nc.gpsimd.partition_broadcast(rbc[:], rec[:])
                    nc.vector.tensor_mul(
                        oT_sb[hr : hr + 64, hp, c0 : c0 + 512],
                        po[h][64:128, :],
                        rbc[:],
                    )

            # ---- HAM warmup: ~5us of dense dummy matmuls at t=0 (PE is
            # otherwise idle waiting on DMA, and would run the first ~30us
            # at the cold 1.2 GHz clock otherwise).
            ps_w = psF.tile([128, 512], f32, tag="f", name="ps_warm")
            for i in range(24):
                nc.tensor.matmul(
                    ps_w[:, 0:512],
                    lhsT=wu_sb[:, 0:128],
                    rhs=wu_sb[:],
                    start=(i == 0),
                    stop=(i == 23),
                )

            # ---- ramp: the qk quarters for chunks (0,0) and (0,1),
            # k-pipelined with the input DMA stream (the per-k matmul rate
            # roughly matches the DMA arrival rate, so the otherwise-idle
            # PE does 4 quarters for free inside the load window).
            psA = psS.tile([128, 2, 512], f32, tag="s", name="rampA")
            psB = psS.tile([128, 2, 512], f32, tag="s", name="rampB")
            for k in range(KT):
                for q2 in range(2):
                    pst = (psA, psB)[q2]
                    for mi, m in enumerate((0, 2)):
                        nc.tensor.matmul(
                            pst[:, mi, :],
                            lhsT=wqk_sb[:, k, ts(m, 128)],
                            rhs=xT_sb[:, k, ts(q2, 512)],
                            start=(k == 0),
                            stop=(k == KT - 1),
                        )
            for q2 in range(2):
                pst = (psA, psB)[q2]
                for mi, m in enumerate((0, 2)):
                    nc.vector.tensor_scalar_add(
                        qkT_sb[:, m, ts(q2, 512)],
                        pst[:, mi, :],
                        bqk_sb[:, m : m + 1],
                    )

            # filler order: roughly when each unit is first needed; ensure()
            # guarantees correctness if the pace falls behind.
            filler.add(("v", 0), gen_v_quarter(0))
            filler.add(("v", 1), gen_v_quarter(1))
            filler.add(("v", 2), gen_v_quarter(2))
            filler.add(("v", 3), gen_v_quarter(3))
            filler.add(("qk", 0, 2), gen_qk_quarter(0, 2))
            filler.add(("qk", 2, 2), gen_qk_quarter(2, 2))
            filler.add(("v", 4), gen_v_quarter(4))
            filler.add(("v", 5), gen_v_quarter(5))
            filler.add(("qk", 0, 3), gen_qk_quarter(0, 3))
            filler.add(("qk", 2, 3), gen_qk_quarter(2, 3))
            filler.add(("v", 6), gen_v_quarter(6))
            filler.add(("v", 7), gen_v_quarter(7))
            for q2 in range(4):
                filler.add(("qk", 1, q2), gen_qk_quarter(1, q2))
                filler.add(("qk", 3, q2), gen_qk_quarter(3, q2))

            for jj in range(4):
                pf = (
                    [("qk", 0, jj + 1), ("qk", 2, jj + 1)]
                    if jj < 3
                    else [("qk", 1, 0), ("qk", 3, 0)]
                )
                attn_chunk(0, jj, pull_scale=(2.0 if jj < 2 else 1.25), prefetch=pf)
            for jj in range(4):
                pf = (
                    [("qk", 1, jj + 1), ("qk", 3, jj + 1)] if jj < 3 else []
                )
                attn_chunk(1, jj, pull_scale=1.25, prefetch=pf)
                for m in range(4 * jj, 4 * jj + 4):
                    for nch in range(2):
                        filler.add(("proj", m, nch), gen_proj(m, nch))
            filler.drain()

    nc.finalize()
    _module_cache["nc"] = nc
    return nc


def _shard_inputs(x, w_qkv, b_qkv, w_proj):
    """Per-core input dicts. Core c: batch c//4, heads 4*(c%4) .. 4*(c%4)+3."""
    bf = ml_dtypes.bfloat16
    in_maps = []
    xTs = [np.ascontiguousarray(x[b].T).astype(bf) for b in range(B)]
    for c in range(N_CORES):
        b = c // GROUPS
        g = c % GROUPS
        qc = slice(256 * g, 256 * g + 256)
        kc = slice(D + 256 * g, D + 256 * g + 256)
        vc = slice(2 * D + 256 * g, 2 * D + 256 * g + 256)
        # 1/sqrt(hd) scale folded into the q columns of W and into b_q
        w_qk = np.ascontiguousarray(
            np.concatenate([w_qkv[:, qc] * SCALE, w_qkv[:, kc]], axis=1)
        ).astype(bf)
        bq = np.concatenate([b_qkv[qc] * SCALE, b_qkv[kc]]).astype(np.float32)
        b_qk = np.ascontiguousarray(bq.reshape(4, 128).T)
        w_v = np.ascontiguousarray(w_qkv[:, vc]).astype(bf)
        b_v = np.ascontiguousarray(np.broadcast_to(b_qkv[vc], (128, 256))).astype(
            np.float32
        )
        w_pr = np.ascontiguousarray(w_proj[256 * g : 256 * g + 256, :]).astype(bf)
        in_maps.append(
            {
                "xT": xTs[b],
                "w_qk": w_qk,
                "b_qk": b_qk,
                "w_v": w_v,
                "b_v": b_v,
                "w_pr": w_pr,
            }
        )
    return in_maps


def kernel(x, w_qkv, b_qkv, w_proj, b_proj, _spmd_kwargs=None):
    from concourse.bass_utils import run_bass_kernel_spmd

    x = np.asarray(x, dtype=np.float32)
    w_qkv = np.asarray(w_qkv, dtype=np.float32)
    b_qkv = np.asarray(b_qkv, dtype=np.float32)
    w_proj = np.asarray(w_proj, dtype=np.float32)
    b_proj = np.asarray(b_proj, dtype=np.float32)

    nc = _build_module()
    in_maps = _shard_inputs(x, w_qkv, b_qkv, w_proj)
    res = run_bass_kernel_spmd(
        nc, in_maps, list(range(N_CORES)), **(_spmd_kwargs or {})
    )
    out = np.empty((B, S, D), dtype=np.float32)
    for b in range(B):
        acc = np.zeros((S, D), dtype=np.float64)
        for g in range(GROUPS):
            acc += np.asarray(res.results[b * GROUPS + g]["y"], dtype=np.float64)
        out[b] = (acc + b_proj.astype(np.float64)).astype(np.float32)
    if _spmd_kwargs:
        kernel.last_result = res
    return out

